# revision 5
# baseline (speedup 1.0000x reference)
"""ConvLEM Trainium2 kernel.

Data-parallel over batch: B=8 -> one batch element per NeuronCore.
Per core, runs the T=50 step LEM recurrence with three 3x3 convs per step:
  convX: 32->128 (precomputable per step), convY: 32->96, convZ: 32->32.

Layout: channel-major fp16, spatially padded to 66x68 (zero halo ring, even
column alignment for DVE 2x mode). Conv taps are AP offsets into the padded
buffer; the 3 ky taps are handled by stacking 3 row-shifted copies of the
input across partition groups (K=96), one matmul per kx tap, N=512 per PSUM
bank. convX+convY accumulate into one [128, HW] PSUM region so the gate sums
xg+yg come out of the PE directly; convZ accumulates onto the xy rows
(partitions 96:128). ACT applies sigmoid/tanh straight from PSUM (conv biases
folded into the ACT bias), DVE does the state lerps in fp16.
"""

import sys

sys.path.insert(0, "/opt/trn_rl_repo")

import numpy as np

import concourse.bass as bass  # noqa: F401  (import side effects / dtypes)
import concourse.tile as tile
from concourse import bacc, mybir
from concourse.bass_utils import run_bass_kernel_spmd

# Problem dims (hardcoded per spec)
B = 8
CIN = 32
COUT = 32
T_FULL = 50
H = 64
W = 64
HP = H + 2  # padded rows
WP = W + 4  # padded cols (2 left, 2 right for 4B alignment of interior)
XOFF = 2  # interior col offset within padded row
NBANK = 8  # psum banks = row-groups of 8
RG = H // NBANK  # rows per bank-slab

FP16 = mybir.dt.float16
FP32 = mybir.dt.float32


def build_program(T: int):
    """Build the Bass program for T recurrence steps. Returns compiled nc."""
    nc = bacc.Bacc("TRN2", target_bir_lowering=False, debug=False, num_devices=B)

    xp_d = nc.dram_tensor("xp", [T, CIN, HP, WP], FP16, kind="ExternalInput")
    wx_d = nc.dram_tensor("wx", [96, 3 * 128], FP16, kind="ExternalInput")
    wy_d = nc.dram_tensor("wy", [96, 3 * 96], FP16, kind="ExternalInput")
    wz_d = nc.dram_tensor("wz", [96, 3 * 32], FP16, kind="ExternalInput")
    bias_d = nc.dram_tensor("bias", [128, 1], FP32, kind="ExternalInput")
    out_d = nc.dram_tensor("out", [T, COUT, H, W], FP16, kind="ExternalOutput")

    xp = xp_d.ap()
    out = out_d.ap()

    with tile.TileContext(nc) as tc:
        with (
            tc.tile_pool(name="persist", bufs=1) as persist,
            tc.tile_pool(name="xs", bufs=3) as xs_pool,
            tc.tile_pool(name="gates", bufs=2) as gates,
            tc.tile_pool(name="tmp", bufs=2) as tmp,
            tc.tile_pool(name="psum", bufs=1, space="PSUM") as psum,
        ):
            # Persistent state: stacked (3x row-shifted) padded Y and Z.
            # Group g (partitions 32g:32g+32) holds data shifted by (g-1) rows:
            #   S[g][c, f] = M[c, f + (g-1)*WP], M = middle group (g=1).
            ys = persist.tile([96, HP, WP], FP16)
            zs = persist.tile([96, HP, WP], FP16)
            wx = persist.tile([96, 3 * 128], FP16)
            wy = persist.tile([96, 3 * 96], FP16)
            wz = persist.tile([96, 3 * 32], FP16)
            bias = persist.tile([128, 1], FP32)
            ps = psum.tile([128, H, W], FP32)  # 8 banks: rows 8b..8b+8 <-> bank b

            nc.sync.dma_start(wx[:], wx_d.ap())
            nc.sync.dma_start(wy[:], wy_d.ap())
            nc.sync.dma_start(wz[:], wz_d.ap())
            nc.sync.dma_start(bias[:], bias_d.ap())
            nc.vector.memset(ys[:], 0.0)
            nc.vector.memset(zs[:], 0.0)

            def conv_mms(w_tile, m_cout, src, p0, start, stop, skip=False):
                """9-tap conv via 3 matmuls (kx taps) x 8 banks, K=96.
                Accumulates into ps[p0:p0+m_cout, :, :]."""
                for kx in range(3):
                    dx = kx - 1
                    lhsT = w_tile[:, kx * m_cout : (kx + 1) * m_cout]
                    for b in range(NBANK):
                        rhs = src[
                            0:96,
                            1 + RG * b : 1 + RG * (b + 1),
                            XOFF + dx : XOFF + dx + W,
                        ]
                        nc.tensor.matmul(
                            ps[p0 : p0 + m_cout, RG * b : RG * (b + 1), :],
                            lhsT,
                            rhs,
                            start=(start and kx == 0),
                            stop=(stop and kx == 2),
                            skip_group_check=skip,
                            tile_position=(0, p0),
                        )

            for t in range(T):
                # ---- X load: 3 row-shifted copies straight from HBM ----
                xs = xs_pool.tile([96, HP, WP], FP16)
                nc.sync.dma_start(xs[0:32, 1:HP, :], xp[t, :, 0 : HP - 1, :])
                nc.sync.dma_start(xs[32:64, :, :], xp[t, :, :, :])
                nc.sync.dma_start(xs[64:96, 0 : HP - 1, :], xp[t, :, 1:HP, :])

                # ---- phase A: convX + convY accumulate into ps ----
                conv_mms(wx, 128, xs, 0, start=True, stop=False)
                conv_mms(wy, 96, ys, 0, start=False, stop=True)

                # ---- gates ----
                # m1 @ base 0, m2 @ base 32 (natural sigma outputs);
                # tz cross-written to base 0 to join the Z cluster.
                m = gates.tile([64, H, W], FP16)  # m1 rows 0:32, m2 rows 32:64
                tz = gates.tile([32, H, W], FP16)
                for h in range(2):
                    rs = slice(32 * h, 32 * (h + 1))
                    nc.scalar.activation(
                        m[:, rs, :],
                        ps[0:64, rs, :],
                        mybir.ActivationFunctionType.Sigmoid,
                        bias=bias[0:64],
                    )
                    nc.scalar.activation(
                        tz[:, rs, :],
                        ps[64:96, rs, :],
                        mybir.ActivationFunctionType.Tanh,
                        bias=bias[64:96],
                    )

                # ---- Z update: Z += m1*(tz - Z) (interior only) ----
                # Z stack: mid = group 0 (ky order 1,0,2) so the cluster
                # {m1, tz, zmid} all sit at base partition 0.
                for h in range(2):
                    rs = slice(32 * h, 32 * (h + 1))
                    zm = zs[0:32, 1 + 32 * h : 1 + 32 * (h + 1), XOFF : XOFF + W]
                    d1 = tmp.tile([32, 32, W], FP16)
                    e1 = tmp.tile([32, 32, W], FP16)
                    nc.vector.tensor_sub(d1[:], tz[:, rs, :], zm)
                    nc.vector.tensor_mul(e1[:], m[0:32, rs, :], d1[:])
                    nc.vector.tensor_add(zm, zm, e1[:])

                # refresh shifted copies of Z (groups: 0=ky1/mid, 1=ky0, 2=ky2)
                nc.sync.dma_start(
                    zs[32:64, 1:HP, :], zs[0:32, 0 : HP - 1, :]
                )
                nc.sync.dma_start(
                    zs[64:96, 0 : HP - 1, :], zs[0:32, 1:HP, :]
                )

                # ---- phase B: convZ accumulates onto xy rows (96:128) ----
                conv_mms(wz, 32, zs, 96, start=False, stop=True, skip=True)

                # th cross-written to base 32 to join the Y cluster
                # {m2, th, ymid} (Y stack mid = group 1, natural ky order).
                th64 = gates.tile([64, H, W], FP16)
                th = th64[32:64]
                for h in range(2):
                    rs = slice(32 * h, 32 * (h + 1))
                    nc.scalar.activation(
                        th[:, rs, :],
                        ps[96:128, rs, :],
                        mybir.ActivationFunctionType.Tanh,
                        bias=bias[96:128],
                    )

                # ---- Y update: Y += m2*(th - Y) ----
                for h in range(2):
                    rs = slice(32 * h, 32 * (h + 1))
                    ym = ys[32:64, 1 + 32 * h : 1 + 32 * (h + 1), XOFF : XOFF + W]
                    d2t = tmp.tile([64, 32, W], FP16)
                    e2t = tmp.tile([64, 32, W], FP16)
                    d2 = d2t[32:64]
                    e2 = e2t[32:64]
                    nc.vector.tensor_sub(d2[:], th[:, rs, :], ym)
                    nc.vector.tensor_mul(e2[:], m[32:64, rs, :], d2[:])
                    nc.vector.tensor_add(ym, ym, e2[:])

                # store Y_new for this step
                nc.sync.dma_start(
                    out[t], ys[32:64, 1 : 1 + H, XOFF : XOFF + W]
                )
                # refresh shifted copies of Y
                nc.sync.dma_start(
                    ys[0:32, 1:HP, :], ys[32:64, 0 : HP - 1, :]
                )
                nc.sync.dma_start(
                    ys[64:96, 0 : HP - 1, :], ys[32:64, 1:HP, :]
                )

    nc.compile()
    return nc


def prep_inputs(X, Wx, bx, Wy, by, Wz, bz, T):
    """Host-side prep -> per-core input maps."""
    # Padded X per core: [T, CIN, HP, WP] fp16
    maps = []
    # weights: lhsT[kx][32*g + ci, co] = W[co, ci, ky_order[g], kx]
    def packw(Wm, m, ky_order=(0, 1, 2)):
        a = Wm.astype(np.float32).transpose(2, 1, 0, 3)  # [ky, ci, co, kx]
        a = a[list(ky_order)]
        cols = [a[:, :, :, kx].reshape(96, m) for kx in range(3)]
        return np.concatenate(cols, axis=1).astype(np.float16)

    wx = packw(Wx, 128)
    wy = packw(Wy, 96)
    wz = packw(Wz, 32, ky_order=(1, 0, 2))  # Z stack: mid group first
    bias = np.zeros((128, 1), np.float32)
    bias[0:96, 0] = bx[0:96] + by[0:96]
    bias[96:128, 0] = bx[96:128] + bz

    for b in range(B):
        xpad = np.zeros((T, CIN, HP, WP), np.float16)
        xpad[:, :, 1 : 1 + H, XOFF : XOFF + W] = (
            X[b, :, :T].transpose(1, 0, 2, 3).astype(np.float16)
        )
        maps.append(
            {"xp": xpad, "wx": wx, "wy": wy, "wz": wz, "bias": bias}
        )
    return maps


_CACHE = {}


def _get_program(T):
    if T not in _CACHE:
        _CACHE[T] = build_program(T)
    return _CACHE[T]


def kernel(X, Wx, bx, Wy, by, Wz, bz, _T=None, _trace=False):
    T = _T or X.shape[2]
    nc = _get_program(T)
    in_maps = prep_inputs(X, Wx, bx, Wy, by, Wz, bz, T)
    res = run_bass_kernel_spmd(
        nc, in_maps, core_ids=list(range(B)), trace=_trace
    )
    outs = []
    for b in range(B):
        o = res.results[b]["out"].astype(np.float32)  # [T, COUT, H, W]
        outs.append(o.transpose(1, 0, 2, 3))  # [COUT, T, H, W]
    full = np.stack(outs, axis=0)  # [B, COUT, T, H, W]
    if _trace:
        kernel._last_results = res
    return full


# revision 11
# speedup vs baseline: 10.0913x; 10.0913x over previous
"""ConvLEM Trainium2 kernel (v2: quarter-pipelined, convX lookahead).

Data-parallel over batch: B=8 -> one batch element per NeuronCore.
Per core, runs the T=50 step LEM recurrence with three 3x3 convs per step.

Layout: channel-major fp16, spatially padded to 66x68 (zero halo ring, even
column alignment for DVE 2x mode). Conv taps are AP offsets into the padded
buffer; the 3 ky taps come from 3 row-shifted copies of the input stacked
across partition groups (K=96), one matmul per kx tap, N=512 per PSUM bank.

Per step, per quarter-image (16 rows = 2 PSUM banks):
  convX (issued during the PREVIOUS step, PE gap filler) + convY accumulate
  into one [128, 16, 64] PSUM tile; one sigmoid over rows 0:96 produces
  m1, m2 and v (tanh via 2*sigmoid(2x)-1, folded into the DVE chain);
  Z lerp on DVE (scalar_tensor_tensor x2 + add); convZ accumulates onto the
  xy rows (96:128, col-group tile_position); second sigmoid; Y lerp.
Conv biases (and the x2 tanh prescale) fold into the ACT bias/scale vectors.
States Y/Z live in SBUF; their ky-shifted stacks are refreshed by 2 half-image
SBUF->SBUF DMAs per tensor per step. X / output stream over gpsimd (SWDGE)
to keep the sync-engine HWDGE free for the latency-critical copies.
Output is stored padded (contiguous DMA) and sliced on host.
"""

import sys

sys.path.insert(0, "/opt/trn_rl_repo")

import numpy as np

import concourse.bass as bass  # noqa: F401
import concourse.tile as tile
from concourse import bacc, mybir
from concourse.bass_utils import run_bass_kernel_spmd

B = 8
CIN = 32
COUT = 32
T_FULL = 50
H = 64
W = 64
HP = H + 2
WP = W + 4
XOFF = 2
NQ = 4  # quarters per image
QR = H // NQ  # 16 rows per quarter
RG = 8  # rows per psum bank

FP16 = mybir.dt.float16
FP32 = mybir.dt.float32
SIG = mybir.ActivationFunctionType.Sigmoid
ALU = mybir.AluOpType


def build_program(T: int):
    nc = bacc.Bacc("TRN2", target_bir_lowering=False, debug=False, num_devices=B)

    xp_d = nc.dram_tensor("xp", [T, CIN, HP, WP], FP16, kind="ExternalInput")
    wx_d = nc.dram_tensor("wx", [96, 3 * 128], FP16, kind="ExternalInput")
    wy_d = nc.dram_tensor("wy", [96, 3 * 96], FP16, kind="ExternalInput")
    wz_d = nc.dram_tensor("wz", [96, 3 * 32], FP16, kind="ExternalInput")
    bias_d = nc.dram_tensor("bias", [128, 1], FP32, kind="ExternalInput")
    scale_d = nc.dram_tensor("scale", [128, 1], FP32, kind="ExternalInput")
    out_d = nc.dram_tensor("out", [T, COUT, HP, WP], FP16, kind="ExternalOutput")

    xp = xp_d.ap()
    out = out_d.ap()

    with tile.TileContext(nc) as tc:
        with (
            tc.tile_pool(name="persist", bufs=1) as persist,
            tc.tile_pool(name="xs", bufs=3) as xs_pool,
            tc.tile_pool(name="mv", bufs=6) as mv_pool,
            tc.tile_pool(name="v2", bufs=3) as v2_pool,
            tc.tile_pool(name="t32", bufs=3) as t32,
            tc.tile_pool(name="t96", bufs=3) as t96,
            tc.tile_pool(name="t64a", bufs=3) as t64a,
            tc.tile_pool(name="t64b", bufs=3) as t64b,
            tc.tile_pool(name="psq", bufs=4, space="PSUM") as psq,
        ):
            # Y stack: natural ky order (0,1,2), mid = group 1 (base 32).
            # Z stack: ky order (0,2,1), mid = group 2 (base 64) so the
            # Z cluster {v@64, zmid@64} shares a base; m1 crosses in via d1@0.
            ys = persist.tile([96, HP, WP], FP16)
            zs = persist.tile([96, HP, WP], FP16)
            wx = persist.tile([96, 3 * 128], FP16)
            wy = persist.tile([96, 3 * 96], FP16)
            wz = persist.tile([96, 3 * 32], FP16)
            bias = persist.tile([128, 1], FP32)
            scale = persist.tile([128, 1], FP32)

            nc.sync.dma_start(wx[:], wx_d.ap())
            nc.sync.dma_start(wy[:], wy_d.ap())
            nc.sync.dma_start(wz[:], wz_d.ap())
            nc.sync.dma_start(bias[:], bias_d.ap())
            nc.sync.dma_start(scale[:], scale_d.ap())
            nc.vector.memset(ys[:], 0.0)
            nc.vector.memset(zs[:], 0.0)

            def load_x(t, xs):
                # 3 row-shifted copies straight from HBM (SWDGE / Pool)
                nc.gpsimd.dma_start(xs[0:32, 1:HP, :], xp[t, :, 0 : HP - 1, :])
                nc.gpsimd.dma_start(xs[32:64, :, :], xp[t, :, :, :])
                nc.gpsimd.dma_start(xs[64:96, 0 : HP - 1, :], xp[t, :, 1:HP, :])

            def conv_q(ps_q, w_tile, m_cout, src, q, p0, start, stop, skip=False):
                """One conv over quarter q: 3 kx taps x 2 banks, K=96."""
                for kx in range(3):
                    dx = kx - 1
                    lhsT = w_tile[:, kx * m_cout : (kx + 1) * m_cout]
                    for hb in range(2):
                        r0 = QR * q + RG * hb  # unpadded row of bank start
                        rhs = src[
                            0:96,
                            1 + r0 : 1 + r0 + RG,
                            XOFF + dx : XOFF + dx + W,
                        ]
                        nc.tensor.matmul(
                            ps_q[p0 : p0 + m_cout, RG * hb : RG * (hb + 1), :],
                            lhsT,
                            rhs,
                            start=(start and kx == 0),
                            stop=(stop and kx == 2),
                            skip_group_check=skip,
                            tile_position=(0, p0),
                        )

            def conv_x(t, xs):
                tiles = []
                for q in range(NQ):
                    ps_f = psq.tile([128, QR * W], FP32, name="ps_f", tag="ps")
                    ps_q = ps_f.rearrange("p (r c) -> p r c", c=W)
                    conv_q(ps_q, wx, 128, xs, q, 0, start=True, stop=False)
                    tiles.append(ps_q)
                return tiles

            def zrow(q):  # interior rows of quarter q in padded coords
                return slice(1 + QR * q, 1 + QR * (q + 1))

            # prologue: X for steps 0 and 1, convX for step 0
            xs_cur = xs_pool.tile([96, HP, WP], FP16)
            load_x(0, xs_cur)
            cx = conv_x(0, xs_cur)
            if T > 1:
                xs_nxt = xs_pool.tile([96, HP, WP], FP16)
                load_x(1, xs_nxt)

            for t in range(T):
                mvs = []
                for q in range(NQ):
                    ps_q = cx[q]
                    conv_q(ps_q, wy, 96, ys, q, 0, start=False, stop=True)
                    mv = mv_pool.tile([96, QR, W], FP16)
                    nc.scalar.activation(
                        mv[:], ps_q[0:96], SIG, bias=bias[0:96], scale=scale[0:96]
                    )
                    mvs.append(mv)
                    # Z lerp: Z += m1*(2v-1-Z), v@64, zmid@64, m1@0
                    zm = zs[64:96, zrow(q), XOFF : XOFF + W]
                    d1 = t32.tile([32, QR, W], FP16)
                    e1 = t96.tile([96, QR, W], FP16, name="e1", tag="e1")[64:96]
                    nc.vector.scalar_tensor_tensor(
                        d1[:], mv[64:96], 2.0, zm, ALU.mult, ALU.subtract
                    )
                    nc.vector.scalar_tensor_tensor(
                        e1[:], d1[:], 1.0, mv[0:32], ALU.subtract, ALU.mult
                    )
                    nc.vector.tensor_add(zm, zm, e1[:])
                    # Shifted-copy refresh. convZ(q) reads stacked rows that
                    # touch Z_new of quarter q+1's first row, so copy A fires
                    # after Zup(q2) (covers convZ q0,q1: stacked rows <=32),
                    # copy B after Zup(q3).
                    if q == 2:
                        # S0[r] = M[r-1], rows 1..33; S1[r] = M[r+1], rows 0..32
                        nc.sync.dma_start(zs[0:32, 1:34, :], zs[64:96, 0:33, :])
                        nc.sync.dma_start(zs[32:64, 0:33, :], zs[64:96, 1:34, :])
                    elif q == 3:
                        nc.sync.dma_start(zs[0:32, 34:HP, :], zs[64:96, 33 : HP - 1, :])
                        nc.sync.dma_start(
                            zs[32:64, 33 : HP - 1, :], zs[64:96, 34:HP, :]
                        )

                for q in range(NQ):
                    ps_q = cx[q]
                    conv_q(ps_q, wz, 32, zs, q, 96, start=False, stop=True, skip=True)
                    v2 = v2_pool.tile([64, QR, W], FP16, name="v2", tag="v2")[32:64]
                    nc.scalar.activation(
                        v2[:], ps_q[96:128], SIG, bias=bias[96:128], scale=scale[96:128]
                    )
                    # Y lerp: Y += m2*(2v2-1-Y), all @32
                    ym = ys[32:64, zrow(q), XOFF : XOFF + W]
                    d2 = t64a.tile([64, QR, W], FP16, name="d2", tag="d2")[32:64]
                    e2 = t64b.tile([64, QR, W], FP16, name="e2", tag="e2")[32:64]
                    nc.vector.scalar_tensor_tensor(
                        d2[:], v2[:], 2.0, ym, ALU.mult, ALU.subtract
                    )
                    nc.vector.scalar_tensor_tensor(
                        e2[:], d2[:], 1.0, mvs[q][32:64], ALU.subtract, ALU.mult
                    )
                    nc.vector.tensor_add(ym, ym, e2[:])
                    # Y stack refresh (mid @32): same boundary rule for
                    # convY(t+1, q).
                    if q == 2:
                        nc.sync.dma_start(ys[0:32, 1:34, :], ys[32:64, 0:33, :])
                        nc.sync.dma_start(ys[64:96, 0:33, :], ys[32:64, 1:34, :])
                    elif q == 3:
                        nc.sync.dma_start(ys[0:32, 34:HP, :], ys[32:64, 33 : HP - 1, :])
                        nc.sync.dma_start(
                            ys[64:96, 33 : HP - 1, :], ys[32:64, 34:HP, :]
                        )
                    # convX for t+1 as PE gap filler
                    if t + 1 < T:
                        if q == 0:
                            cx_next = []
                        ps_nf = psq.tile([128, QR * W], FP32, name="ps_nf", tag="ps")
                        ps_n = ps_nf.rearrange("p (r c) -> p r c", c=W)
                        conv_q(ps_n, wx, 128, xs_nxt, q, 0, start=True, stop=False)
                        cx_next.append(ps_n)

                # store padded Y (contiguous); host slices the halo off
                nc.gpsimd.dma_start(out[t], ys[32:64, :, :])

                if t + 1 < T:
                    cx = cx_next
                    xs_cur = xs_nxt
                    if t + 2 < T:
                        xs_nxt = xs_pool.tile([96, HP, WP], FP16)
                        load_x(t + 2, xs_nxt)

    nc.compile()
    return nc


def prep_inputs(X, Wx, bx, Wy, by, Wz, bz, T):
    # weights: lhsT[kx][32*g + ci, co] = W[co, ci, ky_order[g], kx]
    def packw(Wm, m, ky_order=(0, 1, 2)):
        a = Wm.astype(np.float32).transpose(2, 1, 0, 3)  # [ky, ci, co, kx]
        a = a[list(ky_order)]
        cols = [a[:, :, :, kx].reshape(96, m) for kx in range(3)]
        return np.concatenate(cols, axis=1).astype(np.float16)

    wx = packw(Wx, 128)
    wy = packw(Wy, 96)
    wz = packw(Wz, 32, ky_order=(0, 2, 1))  # Z stack: mid = group 2

    bias = np.zeros((128, 1), np.float32)
    bias[0:64, 0] = (bx[0:64] + by[0:64]).astype(np.float32)
    bias[64:96, 0] = 2.0 * (bx[64:96] + by[64:96]).astype(np.float32)
    bias[96:128, 0] = 2.0 * (bx[96:128] + bz).astype(np.float32)
    scale = np.ones((128, 1), np.float32)
    scale[64:128, 0] = 2.0

    maps = []
    for b in range(B):
        xpad = np.zeros((T, CIN, HP, WP), np.float16)
        xpad[:, :, 1 : 1 + H, XOFF : XOFF + W] = (
            X[b, :, :T].transpose(1, 0, 2, 3).astype(np.float16)
        )
        maps.append(
            {"xp": xpad, "wx": wx, "wy": wy, "wz": wz, "bias": bias, "scale": scale}
        )
    return maps


_CACHE = {}


def _get_program(T):
    if T not in _CACHE:
        _CACHE[T] = build_program(T)
    return _CACHE[T]


def kernel(X, Wx, bx, Wy, by, Wz, bz, _T=None, _trace=False):
    T = _T or X.shape[2]
    nc = _get_program(T)
    in_maps = prep_inputs(X, Wx, bx, Wy, by, Wz, bz, T)
    res = run_bass_kernel_spmd(nc, in_maps, core_ids=list(range(B)), trace=_trace)
    outs = []
    for b in range(B):
        o = res.results[b]["out"].astype(np.float32)  # [T, COUT, HP, WP]
        o = o[:, :, 1 : 1 + H, XOFF : XOFF + W]
        outs.append(o.transpose(1, 0, 2, 3))  # [COUT, T, H, W]
    full = np.stack(outs, axis=0)  # [B, COUT, T, H, W]
    if _trace:
        kernel._last_results = res
    return full


# revision 12
# speedup vs baseline: 13.6142x; 1.3491x over previous
"""ConvLEM Trainium2 kernel (v2: quarter-pipelined, convX lookahead).

Data-parallel over batch: B=8 -> one batch element per NeuronCore.
Per core, runs the T=50 step LEM recurrence with three 3x3 convs per step.

Layout: channel-major fp16, spatially padded to 66x68 (zero halo ring, even
column alignment for DVE 2x mode). Conv taps are AP offsets into the padded
buffer; the 3 ky taps come from 3 row-shifted copies of the input stacked
across partition groups (K=96), one matmul per kx tap, N=512 per PSUM bank.

Per step, per quarter-image (16 rows = 2 PSUM banks):
  convX (issued during the PREVIOUS step, PE gap filler) + convY accumulate
  into one [128, 16, 64] PSUM tile; one sigmoid over rows 0:96 produces
  m1, m2 and v (tanh via 2*sigmoid(2x)-1, folded into the DVE chain);
  Z lerp on DVE (scalar_tensor_tensor x2 + add); convZ accumulates onto the
  xy rows (96:128, col-group tile_position); second sigmoid; Y lerp.
Conv biases (and the x2 tanh prescale) fold into the ACT bias/scale vectors.
States Y/Z live in SBUF; their ky-shifted stacks are refreshed by 2 half-image
SBUF->SBUF DMAs per tensor per step. X / output stream over gpsimd (SWDGE)
to keep the sync-engine HWDGE free for the latency-critical copies.
Output is stored padded (contiguous DMA) and sliced on host.
"""

import sys

sys.path.insert(0, "/opt/trn_rl_repo")

import numpy as np

import concourse.bass as bass  # noqa: F401
import concourse.tile as tile
from concourse import bacc, mybir
from concourse.bass_utils import run_bass_kernel_spmd

B = 8
CIN = 32
COUT = 32
T_FULL = 50
H = 64
W = 64
HP = H + 2
WP = W + 4
XOFF = 2
NQ = 4  # quarters per image
QR = H // NQ  # 16 rows per quarter
RG = 8  # rows per psum bank

FP16 = mybir.dt.float16
FP32 = mybir.dt.float32
SIG = mybir.ActivationFunctionType.Sigmoid
TANH = mybir.ActivationFunctionType.Tanh
ALU = mybir.AluOpType


def build_program(T: int):
    nc = bacc.Bacc("TRN2", target_bir_lowering=False, debug=False, num_devices=B)

    xp_d = nc.dram_tensor("xp", [T, CIN, HP, WP], FP16, kind="ExternalInput")
    wx_d = nc.dram_tensor("wx", [96, 3 * 128], FP16, kind="ExternalInput")
    wy_d = nc.dram_tensor("wy", [96, 3 * 96], FP16, kind="ExternalInput")
    wz_d = nc.dram_tensor("wz", [96, 3 * 32], FP16, kind="ExternalInput")
    bias_d = nc.dram_tensor("bias", [128, 1], FP32, kind="ExternalInput")
    scale_d = nc.dram_tensor("scale", [128, 1], FP32, kind="ExternalInput")
    out_d = nc.dram_tensor("out", [T, COUT, HP, WP], FP16, kind="ExternalOutput")

    xp = xp_d.ap()
    out = out_d.ap()

    with tile.TileContext(nc) as tc:
        with (
            tc.tile_pool(name="persist", bufs=1) as persist,
            tc.tile_pool(name="xs", bufs=3) as xs_pool,
            tc.tile_pool(name="mv", bufs=6) as mv_pool,
            tc.tile_pool(name="v2", bufs=3) as v2_pool,
            tc.tile_pool(name="t32", bufs=3) as t32,
            tc.tile_pool(name="t96", bufs=3) as t96,
            tc.tile_pool(name="t64a", bufs=3) as t64a,
            tc.tile_pool(name="t64b", bufs=3) as t64b,
            tc.tile_pool(name="psq", bufs=4, space="PSUM") as psq,
        ):
            # Y stack: natural ky order (0,1,2), mid = group 1 (base 32).
            # Z stack: ky order (0,2,1), mid = group 2 (base 64) so the
            # Z cluster {v@64, zmid@64} shares a base; m1 crosses in via d1@0.
            ys = persist.tile([96, HP, WP], FP16)
            zs = persist.tile([96, HP, WP], FP16)
            wx = persist.tile([96, 3 * 128], FP16)
            wy = persist.tile([96, 3 * 96], FP16)
            wz = persist.tile([96, 3 * 32], FP16)
            bias = persist.tile([128, 1], FP32)
            scale = persist.tile([128, 1], FP32)

            nc.sync.dma_start(wx[:], wx_d.ap())
            nc.sync.dma_start(wy[:], wy_d.ap())
            nc.sync.dma_start(wz[:], wz_d.ap())
            nc.sync.dma_start(bias[:], bias_d.ap())
            nc.sync.dma_start(scale[:], scale_d.ap())
            nc.vector.memset(ys[:], 0.0)
            nc.vector.memset(zs[:], 0.0)

            def load_x(t, xs):
                # 3 row-shifted copies straight from HBM (SWDGE / Pool)
                nc.gpsimd.dma_start(xs[0:32, 1:HP, :], xp[t, :, 0 : HP - 1, :])
                nc.gpsimd.dma_start(xs[32:64, :, :], xp[t, :, :, :])
                nc.gpsimd.dma_start(xs[64:96, 0 : HP - 1, :], xp[t, :, 1:HP, :])

            def conv_q(ps_q, w_tile, m_cout, src, q, p0, start, stop, skip=False):
                """One conv over quarter q: 3 kx taps x 2 banks, K=96."""
                for kx in range(3):
                    dx = kx - 1
                    lhsT = w_tile[:, kx * m_cout : (kx + 1) * m_cout]
                    for hb in range(2):
                        r0 = QR * q + RG * hb  # unpadded row of bank start
                        rhs = src[
                            0:96,
                            1 + r0 : 1 + r0 + RG,
                            XOFF + dx : XOFF + dx + W,
                        ]
                        nc.tensor.matmul(
                            ps_q[p0 : p0 + m_cout, RG * hb : RG * (hb + 1), :],
                            lhsT,
                            rhs,
                            start=(start and kx == 0),
                            stop=(stop and kx == 2),
                            skip_group_check=skip,
                            tile_position=(0, p0),
                        )

            def conv_x(t, xs):
                tiles = []
                for q in range(NQ):
                    ps_f = psq.tile([128, QR * W], FP32, name="ps_f", tag="ps")
                    ps_q = ps_f.rearrange("p (r c) -> p r c", c=W)
                    conv_q(ps_q, wx, 128, xs, q, 0, start=True, stop=False)
                    tiles.append(ps_q)
                return tiles

            def zrow(q):  # interior rows of quarter q in padded coords
                return slice(1 + QR * q, 1 + QR * (q + 1))

            # prologue: X for steps 0 and 1, convX for step 0
            xs_cur = xs_pool.tile([96, HP, WP], FP16)
            load_x(0, xs_cur)
            cx = conv_x(0, xs_cur)
            if T > 1:
                xs_nxt = xs_pool.tile([96, HP, WP], FP16)
                load_x(1, xs_nxt)

            for t in range(T):
                mvs = []
                for q in range(NQ):
                    ps_q = cx[q]
                    conv_q(ps_q, wy, 96, ys, q, 0, start=False, stop=True)
                    mv = mv_pool.tile([96, QR, W], FP16)
                    nc.scalar.activation(
                        mv[0:64], ps_q[0:64], SIG, bias=bias[0:64]
                    )
                    nc.scalar.activation(
                        mv[64:96], ps_q[64:96], TANH, bias=bias[64:96]
                    )
                    mvs.append(mv)
                    # Z lerp: Z += m1*(tz-Z); tz@64, zmid@64, m1@0
                    zm = zs[64:96, zrow(q), XOFF : XOFF + W]
                    d1 = t32.tile([32, QR, W], FP16)
                    e1 = t96.tile([96, QR, W], FP16, name="e1", tag="e1")[64:96]
                    nc.vector.tensor_sub(d1[:], mv[64:96], zm)
                    nc.vector.tensor_mul(e1[:], mv[0:32], d1[:])
                    nc.vector.tensor_add(zm, zm, e1[:])
                    # Shifted-copy refresh. convZ(q) reads stacked rows that
                    # touch Z_new of quarter q+1's first row, so copy A fires
                    # after Zup(q2) (covers convZ q0,q1: stacked rows <=32),
                    # copy B after Zup(q3).
                    if q == 2:
                        # S0[r] = M[r-1], rows 1..33; S1[r] = M[r+1], rows 0..32
                        nc.sync.dma_start(zs[0:32, 1:34, :], zs[64:96, 0:33, :])
                        nc.sync.dma_start(zs[32:64, 0:33, :], zs[64:96, 1:34, :])
                    elif q == 3:
                        nc.sync.dma_start(zs[0:32, 34:HP, :], zs[64:96, 33 : HP - 1, :])
                        nc.sync.dma_start(
                            zs[32:64, 33 : HP - 1, :], zs[64:96, 34:HP, :]
                        )

                for q in range(NQ):
                    ps_q = cx[q]
                    conv_q(ps_q, wz, 32, zs, q, 96, start=False, stop=True, skip=True)
                    th = v2_pool.tile([64, QR, W], FP16, name="th", tag="th")[32:64]
                    nc.scalar.activation(
                        th[:], ps_q[96:128], TANH, bias=bias[96:128]
                    )
                    # Y lerp: Y += m2*(th-Y), all @32
                    ym = ys[32:64, zrow(q), XOFF : XOFF + W]
                    d2 = t64a.tile([64, QR, W], FP16, name="d2", tag="d2")[32:64]
                    e2 = t64b.tile([64, QR, W], FP16, name="e2", tag="e2")[32:64]
                    nc.vector.tensor_sub(d2[:], th[:], ym)
                    nc.vector.tensor_mul(e2[:], mvs[q][32:64], d2[:])
                    nc.vector.tensor_add(ym, ym, e2[:])
                    # Y stack refresh (mid @32): same boundary rule for
                    # convY(t+1, q).
                    if q == 2:
                        nc.sync.dma_start(ys[0:32, 1:34, :], ys[32:64, 0:33, :])
                        nc.sync.dma_start(ys[64:96, 0:33, :], ys[32:64, 1:34, :])
                    elif q == 3:
                        nc.sync.dma_start(ys[0:32, 34:HP, :], ys[32:64, 33 : HP - 1, :])
                        nc.sync.dma_start(
                            ys[64:96, 33 : HP - 1, :], ys[32:64, 34:HP, :]
                        )
                    # convX for t+1 as PE gap filler
                    if t + 1 < T:
                        if q == 0:
                            cx_next = []
                        ps_nf = psq.tile([128, QR * W], FP32, name="ps_nf", tag="ps")
                        ps_n = ps_nf.rearrange("p (r c) -> p r c", c=W)
                        conv_q(ps_n, wx, 128, xs_nxt, q, 0, start=True, stop=False)
                        cx_next.append(ps_n)

                # store padded Y (contiguous); host slices the halo off
                nc.sync.dma_start(out[t], ys[32:64, :, :])

                if t + 1 < T:
                    cx = cx_next
                    xs_cur = xs_nxt
                    if t + 2 < T:
                        xs_nxt = xs_pool.tile([96, HP, WP], FP16)
                        load_x(t + 2, xs_nxt)

    nc.compile()
    return nc


def prep_inputs(X, Wx, bx, Wy, by, Wz, bz, T):
    # weights: lhsT[kx][32*g + ci, co] = W[co, ci, ky_order[g], kx]
    def packw(Wm, m, ky_order=(0, 1, 2)):
        a = Wm.astype(np.float32).transpose(2, 1, 0, 3)  # [ky, ci, co, kx]
        a = a[list(ky_order)]
        cols = [a[:, :, :, kx].reshape(96, m) for kx in range(3)]
        return np.concatenate(cols, axis=1).astype(np.float16)

    wx = packw(Wx, 128)
    wy = packw(Wy, 96)
    wz = packw(Wz, 32, ky_order=(0, 2, 1))  # Z stack: mid = group 2

    bias = np.zeros((128, 1), np.float32)
    bias[0:96, 0] = (bx[0:96] + by[0:96]).astype(np.float32)
    bias[96:128, 0] = (bx[96:128] + bz).astype(np.float32)
    scale = np.ones((128, 1), np.float32)

    maps = []
    for b in range(B):
        xpad = np.zeros((T, CIN, HP, WP), np.float16)
        xpad[:, :, 1 : 1 + H, XOFF : XOFF + W] = (
            X[b, :, :T].transpose(1, 0, 2, 3).astype(np.float16)
        )
        maps.append(
            {"xp": xpad, "wx": wx, "wy": wy, "wz": wz, "bias": bias, "scale": scale}
        )
    return maps


_CACHE = {}


def _get_program(T):
    if T not in _CACHE:
        _CACHE[T] = build_program(T)
    return _CACHE[T]


def kernel(X, Wx, bx, Wy, by, Wz, bz, _T=None, _trace=False):
    T = _T or X.shape[2]
    nc = _get_program(T)
    in_maps = prep_inputs(X, Wx, bx, Wy, by, Wz, bz, T)
    res = run_bass_kernel_spmd(nc, in_maps, core_ids=list(range(B)), trace=_trace)
    outs = []
    for b in range(B):
        o = res.results[b]["out"].astype(np.float32)  # [T, COUT, HP, WP]
        o = o[:, :, 1 : 1 + H, XOFF : XOFF + W]
        outs.append(o.transpose(1, 0, 2, 3))  # [COUT, T, H, W]
    full = np.stack(outs, axis=0)  # [B, COUT, T, H, W]
    if _trace:
        kernel._last_results = res
    return full
